# revision 3
# baseline (speedup 1.0000x reference)
"""CRF log-prob: Bass/Tile kernel on 8 trn2 NeuronCores, data-parallel over batch.

Strategy (wire-bound problem: the axon tunnel moves ~40 MB/s):
  - Emissions are quantized to int8 on the host (32 MB instead of 128 MB f32)
    and the forward algorithm runs on-device in the *linear domain*: each step
    is one 65x65 f32 matmul (PE) + one elementwise multiply (DVE).
  - State is v[j,b] = exp(alpha[j,b] - kappa*t), with the constant kappa folded
    into E' = exp(transitions - kappa). Row 64 of the state is an accumulator
    that captures u_t = sum_j exp(end_j) * v_t[j] at t = len_b - 1; the capture
    schedule, sequence-death masking, and accumulator decay gamma are all
    encoded by the host in the int8 emission stream, so the device program is a
    fixed 2048-step scan with zero control flow.
  - The gold-path score (pure gathers) is computed on the host, overlapped with
    the device call. logprob = score - (kappa*(len-1) + log(acc) - log(gamma)).

The device program is built once and run through the same jit/shard_map path
run_bass_kernel_spmd uses under axon, cached at module level so repeat calls
only pay the H2D transfer + execution.
"""
import os
import sys
import threading

import numpy as np

for _p in ("/root/.axon_site", "/opt/trn_rl_repo", "/root/.axon_site/_ro/trn_rl_repo",
           "/root/.axon_site/_ro/pypackages"):
    if os.path.isdir(_p) and _p not in sys.path:
        sys.path.append(_p)

B, T, N = 256, 2048, 64
M = 8          # cores
BS = B // M    # 32 sequences per core
P = N + 1      # 64 tag rows + 1 accumulator row
W = (T + 1) * BS
CHUNK = 32     # scan steps per For_i iteration

_STATE: dict = {}


# --------------------------------------------------------------------------
# device program
# --------------------------------------------------------------------------

def _build_nc():
    import concourse.bass as bass
    import concourse.mybir as mybir
    from concourse import tile
    from contextlib import ExitStack

    nc = bass.Bass()
    emq = nc.dram_tensor("emq", [P, W], mybir.dt.int8, kind="ExternalInput")
    eaug = nc.dram_tensor("eaug", [P, P], mybir.dt.float32, kind="ExternalInput")
    scol = nc.dram_tensor("scol", [P, 1], mybir.dt.float32, kind="ExternalInput")
    bcol = nc.dram_tensor("bcol", [P, 1], mybir.dt.float32, kind="ExternalInput")
    acc = nc.dram_tensor("acc", [1, BS], mybir.dt.float32, kind="ExternalOutput")

    CW = CHUNK * BS
    with ExitStack() as ctx, tile.TileContext(nc) as tc:
        cpool = ctx.enter_context(tc.tile_pool(name="const", bufs=1))
        emqpool = ctx.enter_context(tc.tile_pool(name="emq", bufs=1))
        xpool = ctx.enter_context(tc.tile_pool(name="expem", bufs=2))
        pspool = ctx.enter_context(tc.tile_pool(name="ps", bufs=4, space="PSUM"))
        vpool = ctx.enter_context(tc.tile_pool(name="state", bufs=1))

        emq_sb = emqpool.tile([P, W], mybir.dt.int8)
        nc.sync.dma_start(emq_sb[:], emq[:])
        eaug_sb = cpool.tile([P, P], mybir.dt.float32)
        nc.sync.dma_start(eaug_sb[:], eaug[:])
        scol_sb = cpool.tile([P, 1], mybir.dt.float32)
        nc.sync.dma_start(scol_sb[:], scol[:])
        bcol_sb = cpool.tile([P, 1], mybir.dt.float32)
        nc.sync.dma_start(bcol_sb[:], bcol[:])

        v = vpool.tile([P, BS], mybir.dt.float32)
        nc.scalar.activation(v[:], emq_sb[:, 0:BS], mybir.ActivationFunctionType.Exp,
                             bias=bcol_sb[:], scale=scol_sb[:])

        with tc.For_i(BS, W, CW) as off:
            ex = xpool.tile([P, CW], mybir.dt.float32)
            nc.scalar.activation(ex[:], emq_sb[:, bass.ds(off, CW)],
                                 mybir.ActivationFunctionType.Exp, scale=scol_sb[:])
            for k in range(CHUNK):
                ps = pspool.tile([P, BS], mybir.dt.float32)
                nc.tensor.matmul(ps[:], eaug_sb[:], v[:], start=True, stop=True)
                nc.vector.tensor_mul(v[:], ps[:], ex[:, k * BS:(k + 1) * BS])

        nc.sync.dma_start(acc[:], v[P - 1:P, :])
    return nc


def _get_runner():
    """jit(shard_map(bass_exec)) over 8 cores — the run_bass_kernel_spmd axon
    execution path, built once and cached so the warm call doesn't retrace."""
    if "runner" in _STATE:
        return _STATE["runner"]
    import jax
    import concourse.mybir as mybir
    from concourse import bass2jax
    from jax.sharding import Mesh, PartitionSpec
    try:
        from jax.experimental.shard_map import shard_map
    except ImportError:
        from jax.sharding import shard_map  # newer jax

    bass2jax.install_neuronx_cc_hook()
    nc = _build_nc()

    in_names, out_names, out_avals = [], [], []
    for alloc in nc.m.functions[0].allocations:
        if not isinstance(alloc, mybir.MemoryLocationSet):
            continue
        name = alloc.memorylocations[0].name
        if alloc.kind == "ExternalInput":
            in_names.append(name)
        elif alloc.kind == "ExternalOutput":
            out_names.append(name)
            out_avals.append(jax.core.ShapedArray(
                tuple(alloc.tensor_shape), mybir.dt.np(alloc.dtype)))
    n_params = len(in_names)
    bind_names = tuple(in_names + out_names)

    def _body(*args):
        outs = bass2jax._bass_exec_p.bind(
            *args,
            out_avals=tuple(out_avals),
            in_names=bind_names,
            out_names=tuple(out_names),
            lowering_input_output_aliases=(),
            sim_require_finite=True,
            sim_require_nnan=True,
            nc=nc,
        )
        return tuple(outs)

    devices = jax.devices()[:M]
    mesh = Mesh(np.asarray(devices), ("core",))
    n_outs = len(out_names)
    fn = jax.jit(
        shard_map(_body, mesh=mesh,
                  in_specs=(PartitionSpec("core"),) * (n_params + n_outs),
                  out_specs=(PartitionSpec("core"),) * n_outs,
                  check_rep=False),
        donate_argnums=tuple(range(n_params, n_params + n_outs)),
        keep_unused=True,
    )
    out_shapes = [tuple(a.shape) for a in out_avals]
    out_dtypes = [a.dtype for a in out_avals]
    _STATE["runner"] = (fn, in_names, out_names, out_shapes, out_dtypes)
    return _STATE["runner"]


# --------------------------------------------------------------------------
# host side
# --------------------------------------------------------------------------

def _host_prep(em, lengths):
    """Build the concatenated [8*65, W] int8 emission stream + aux constants."""
    amax = float(np.abs(em).max())
    s = 2.0 * amax / 127.0 if amax > 0 else 1.0 / 127.0
    inv_s = np.float32(1.0 / s)
    q_g = int(max(-127, round(-5.0 / s)))
    gamma_log = s * q_g
    # kappa: mean per-step log-growth, estimated from a sample; it cancels
    # exactly in the final combine, only the f32 dynamic range depends on it.
    samp = em[:: max(1, B // 16), :: max(1, T // 64)].astype(np.float64)
    kap = float(np.log(np.exp(samp).sum(-1)).mean())

    emq = np.rint(em * inv_s).astype(np.int8)              # |values| <= 64
    dead = np.arange(T)[None, :] >= lengths[:, None]       # [B,T]
    emq[dead] = -128
    k_idx = np.arange(T + 1)[:, None]

    stream = np.empty((M * P, W), dtype=np.int8)
    for c in range(M):
        sl = slice(c * BS, (c + 1) * BS)
        blk = stream[c * P:(c + 1) * P]
        blk[:N, :T * BS] = emq[sl].transpose(2, 1, 0).reshape(N, T * BS)
        blk[:N, T * BS:] = -128
        blk[N, :] = np.where(k_idx <= lengths[None, sl], q_g, 0) \
            .astype(np.int8).reshape(-1)
    return stream, s, kap, gamma_log, q_g


def _host_aux(trans, start, end, s, kap):
    eaug = np.zeros((P, P), dtype=np.float32)
    eaug[:N, :N] = np.exp(trans.astype(np.float64) - kap).astype(np.float32)
    eaug[:N, N] = np.exp(end.astype(np.float32))
    eaug[N, N] = 1.0
    scol = np.full((P, 1), s, dtype=np.float32)
    bcol = np.zeros((P, 1), dtype=np.float32)
    bcol[:N, 0] = start
    bcol[N, 0] = -1e4
    return (np.tile(eaug, (M, 1)), np.tile(scol, (M, 1)), np.tile(bcol, (M, 1)))


def _host_score(em, tags, lengths, trans, start, end):
    mask = np.arange(T)[None, :] < lengths[:, None]
    em_sc = np.take_along_axis(em, tags[..., None], axis=-1)[..., 0]
    trans_sc = trans[tags[:, :-1], tags[:, 1:]]
    last = np.take_along_axis(tags, (lengths - 1)[:, None], axis=1)[:, 0]
    return (start[tags[:, 0]]
            + np.einsum("bt,bt->b", em_sc, mask, dtype=np.float64, casting="unsafe")
            + np.einsum("bt,bt->b", trans_sc, mask[:, 1:], dtype=np.float64,
                        casting="unsafe")
            + end[last])


def _kernel_device(em, tags, lengths, trans, start, end):
    fn, in_names, out_names, out_shapes, out_dtypes = _get_runner()

    stream, s, kap, gamma_log, _ = _host_prep(em, lengths)
    eaug_all, scol_all, bcol_all = _host_aux(trans, start, end, s, kap)
    arr_by_name = {"emq": stream, "eaug": eaug_all, "scol": scol_all,
                   "bcol": bcol_all}
    ins = [arr_by_name[n] for n in in_names]
    zeros = [np.zeros((M * sh[0],) + sh[1:], dt)
             for sh, dt in zip(out_shapes, out_dtypes)]

    result = {}

    def run_device():
        outs = fn(*ins, *zeros)
        result["acc"] = np.asarray(outs[out_names.index("acc")])

    th = threading.Thread(target=run_device)
    th.start()
    score = _host_score(em, tags, lengths, trans, start, end)
    th.join()

    acc = result["acc"].reshape(M * BS).astype(np.float64)
    logz = kap * (lengths - 1) + np.log(acc) - gamma_log
    return (score - logz).astype(np.float32)


def _kernel_numpy(em, tags, lengths, trans, start, end):
    """Fallback: same linear-domain scan on host, full f32 emissions."""
    samp = em[:: max(1, B // 16), :: max(1, T // 64)].astype(np.float64)
    kap = float(np.log(np.exp(samp).sum(-1)).mean())
    Em = np.exp(trans.astype(np.float64) - kap).astype(np.float32)
    expend = np.exp(end.astype(np.float32))
    expem = np.exp(em)                      # [B,T,N] f32
    dead = np.arange(T)[None, :] >= lengths[:, None]
    expem[dead] = 0.0
    v = np.exp(start.astype(np.float32))[None, :] * expem[:, 0]
    u_sel = np.zeros(B, dtype=np.float32)
    take_now = lengths == 1
    u_sel[take_now] = (v[take_now] * expend[None, :]).sum(-1)
    for t in range(1, T):
        v = (v @ Em) * expem[:, t]
        take_now = lengths == t + 1
        if take_now.any():
            u_sel[take_now] = (v[take_now] * expend[None, :]).sum(-1)
    logz = kap * (lengths - 1) + np.log(u_sel.astype(np.float64))
    score = _host_score(em, tags, lengths, trans, start, end)
    return (score - logz).astype(np.float32)


def kernel(emissions, tags, lengths, transitions, start_transitions,
           end_transitions):
    em = np.ascontiguousarray(emissions, dtype=np.float32)
    tags = np.asarray(tags).astype(np.int64)
    lengths = np.asarray(lengths).astype(np.int64)
    trans = np.asarray(transitions, dtype=np.float32)
    start = np.asarray(start_transitions, dtype=np.float32)
    end = np.asarray(end_transitions, dtype=np.float32)
    if not _STATE.get("device_failed"):
        try:
            return _kernel_device(em, tags, lengths, trans, start, end)
        except Exception:
            _STATE["device_failed"] = True
    return _kernel_numpy(em, tags, lengths, trans, start, end)
